# revision 11
# baseline (speedup 1.0000x reference)
"""Trainium2 Bass kernel for context-attention guided top-k masking.

Computes, per sample b:
    scores[n] = cos(ctx[b,n,:], cond[b,:])   (l2-normalized dot product)
    sel       = top_k(scores, k)
    out[b,n,:] = mask_token if n in sel else ctx[b,n,:]

Strategy (pure data parallel over batch, 4 samples per NeuronCore x 8 cores):
  - DMA is the roofline: 2 x 33.5 MiB (ctx in + out) through the serialized
    DMA engines at 360 GB/s ~= 186 us. Everything else must hide under it.
  - Engine balance per [128,512] tile: products x*cond on GpSimd (Pool),
    sum-of-squares via ACT Square+accum, dot-reduce alternating between
    ACT Copy+accum and DVE tensor_reduce, blend via DVE copy_predicated.
  - Selection needs no explicit top-k: find the k-th largest score per
    sample by multisection (7 probes/round, 8 rounds, no hi tracking:
    the interval width is the exact per-round constant wd) on the
    rank-monotone transform g = dot * rsqrt(max(ss, eps^2)) ==
    score * ||cond|| (positive per-sample constant -> identical ranking),
    then mask = g >= tau. Rounds of sample s are emitted interleaved with
    the score tiles of sample s+1 so the serial probe chain hides under
    streaming compute instead of stalling the DVE sequencer.
  - All DMAs issue from the SP sequencer in an explicit order (ins ahead
    of outs, one-sample lookahead bounded by the 23 chunk buffers) so the
    in-order issue stream never starves the DMA engines nor deadlocks on
    tile-pool buffer reuse.
"""

import numpy as np

import concourse.bacc as bacc
import concourse.mybir as mybir
import concourse.tile as tile
from concourse import bass_isa
from concourse import bass_utils

B, N, D = 32, 4096, 512
NCORES = 8
BPC = B // NCORES          # samples per core
TOKP = 128                 # tokens per tile (partition dim)
NT = N // TOKP             # 32 tiles per sample
F32 = mybir.dt.float32
I32 = mybir.dt.int32
Alu = mybir.AluOpType
Act = mybir.ActivationFunctionType

# multisection: probes p_j = lo + j*wd (j=1..7) shrink the interval 8x per
# round; 8 rounds from +-G_HI0 resolve 2*G_HI0/8^8 = 4.8e-7, below the min
# adjacent-score gap (~7e-6 in g-space). |tau| ~ |kth cos| * ||cond|| stays
# well inside +-4 for any realistic input (cos of the k-th is O(0.1)).
BISECT_ITERS = 8
G_HI0 = 4.0
P = 7

MCH = 4                    # tiles per DMA chunk (1 MiB transfers)
NCH = NT // MCH            # 8 chunks per sample
CHUNK_BUFS = 23            # ~2.9 samples in flight (184 KiB/partition)


def _kernel_body(ctx_stack, tc, out_d, ctx_d, cond_d, mt_d, k):
    nc = tc.nc
    kf = float(k)

    const_pool = ctx_stack.enter_context(tc.tile_pool(name="const", bufs=1))
    ctx_pool = ctx_stack.enter_context(
        tc.tile_pool(name="ctx", bufs=CHUNK_BUFS))
    prod_pool = ctx_stack.enter_context(tc.tile_pool(name="prod", bufs=3))
    sq_pool = ctx_stack.enter_context(
        tc.tile_pool(name="sq", bufs=2, space="PSUM"))
    stat_pool = ctx_stack.enter_context(tc.tile_pool(name="stat", bufs=2))
    bis_pool = ctx_stack.enter_context(tc.tile_pool(name="bis", bufs=1))

    # --- constants ---------------------------------------------------------
    js = const_pool.tile([128, P], F32, tag="js")
    for j in range(P):
        nc.vector.memset(js[:, j : j + 1], float(j + 1))

    # mask_token broadcast to [128, D]; cond_feat broadcast per sample
    # (one DMA each; issued first so they're on-chip before first use).
    mtb = const_pool.tile([128, D], F32, tag="mtb")
    nc.sync.dma_start(mtb[:, :], mt_d.unsqueeze(0).partition_broadcast(128))
    cond_bt = const_pool.tile([128, BPC * D], F32, tag="condb")
    nc.sync.dma_start(
        cond_bt[:, :].rearrange("p (s d) -> p s d", s=BPC),
        cond_d.unsqueeze(0).partition_broadcast(128),
    )

    ctx_chunks = {}

    def emit_in(s, c_lo, c_hi):
        src3 = ctx_d[s].rearrange("(t p) d -> p t d", p=TOKP)
        for c in range(c_lo, c_hi):
            ch = ctx_pool.tile([TOKP, MCH * D], F32, tag="cchunk")
            nc.sync.dma_start(
                ch[:, :].rearrange("p (t d) -> p t d", d=D),
                src3[:, c * MCH : (c + 1) * MCH, :],
            )
            ctx_chunks[(s, c)] = ch

    stats = {}

    def emit_score_tile(s, t):
        """Products on Pool; ss on ACT; dot-reduce alternates ACT/DVE."""
        if t == 0:
            stats[s] = (
                stat_pool.tile([128, NT], F32, tag="dots", name="dots"),
                stat_pool.tile([128, NT], F32, tag="ss", name="ss"),
            )
        dots, ss = stats[s]
        ct = ctx_chunks[(s, t // MCH)][:, (t % MCH) * D : (t % MCH + 1) * D]
        scr = prod_pool.tile([TOKP, D], F32, tag="scr")
        # All products on Pool: 1111ns/tile means Pool lags the DMA stream
        # ~12us/sample, but its 142us cumulative still beats the ~145us tail
        # target, and it frees 31us of DVE. Dot-reduce ACT 8 / DVE 24 — the
        # kernel tail is set by each engine's CUMULATIVE work before the
        # last sample's scores, so ACT (whose ss pass is immovable) gets
        # only what keeps its total under ~145us.
        nc.gpsimd.tensor_tensor(
            scr, ct, cond_bt[:, s * D : (s + 1) * D], op=Alu.mult)
        if t % 4 == 0:
            dsc = sq_pool.tile([TOKP, D], F32, tag="dsc")
            nc.scalar.activation(
                dsc[:, :], scr, Act.Copy, accum_out=dots[:, t : t + 1])
        else:
            nc.vector.tensor_reduce(
                dots[:, t : t + 1], scr[:, :],
                op=Alu.add, axis=mybir.AxisListType.X)
        sq = sq_pool.tile([TOKP, D], F32, tag="sqs")
        nc.scalar.activation(
            sq[:, :], ct, Act.Square, accum_out=ss[:, t : t + 1])

    gs = {}

    def emit_gprep(s):
        """g = dot * rsqrt(max(ss, 1e-12)); rsqrt = ACT-sqrt seed + one
        Newton step so the norm factor is ~1e-10-relative accurate."""
        dots, ss = stats[s]
        g2 = stat_pool.tile([128, NT], F32, tag="g2")
        ssc = stat_pool.tile([128, NT], F32, tag="ssc")
        nc.vector.tensor_scalar(ssc[:, :], ss[:, :], 1e-12, None, op0=Alu.max)
        inv = stat_pool.tile([128, NT], F32, tag="inv")
        nc.vector.reciprocal(inv[:, :], ssc[:, :])
        r0 = stat_pool.tile([128, NT], F32, tag="r0")
        nc.scalar.activation(r0[:, :], inv[:, :], Act.Sqrt)
        t2 = stat_pool.tile([128, NT], F32, tag="t2")
        nc.vector.tensor_tensor(t2[:, :], r0[:, :], r0[:, :], op=Alu.mult)
        nc.vector.tensor_tensor(t2[:, :], t2[:, :], ssc[:, :], op=Alu.mult)
        nc.vector.tensor_scalar(t2[:, :], t2[:, :], -0.5, 1.5,
                                op0=Alu.mult, op1=Alu.add)
        nc.vector.tensor_tensor(t2[:, :], t2[:, :], r0[:, :], op=Alu.mult)
        nc.vector.tensor_tensor(g2[:, :], dots[:, :], t2[:, :], op=Alu.mult)
        gs[s] = g2

    bstate = {}

    def emit_bisect_round(s):
        """One multisection round: probes pr_j = lo + j*wd, count g >= pr_j,
        m = #probes with count >= k, lo += m*wd, wd /= 8. The final lo is
        exact: pr_m and the lo update compute fl(fl(m*wd)+lo) identically.
        All state is partition-replicated [128,.] and the cross-partition
        count goes through gpsimd partition_all_reduce, keeping the serial
        probe chain off the PE (cold-p-state matmul round-trips are slow)."""
        if s not in bstate:
            lo = bis_pool.tile([128, 1], F32, tag="lo")
            wd = bis_pool.tile([128, 1], F32, tag="wd")
            nc.vector.memset(lo[:, :], -G_HI0)
            nc.vector.memset(wd[:, :], 2.0 * G_HI0 / (P + 1))
            bstate[s] = (lo, wd)
        lo, wd = bstate[s]
        g2 = gs[s]
        pr = bis_pool.tile([128, P], F32, tag="pr")
        nc.vector.tensor_scalar(pr[:, :], js[:, :], wd[:, 0:1],
                                lo[:, 0:1], op0=Alu.mult, op1=Alu.add)
        cmp = bis_pool.tile([128, P * NT], F32, tag="cmp")
        nc.vector.tensor_tensor(
            cmp[:, :].rearrange("p (j t) -> p j t", j=P),
            g2[:, :].unsqueeze(1).broadcast_to([128, P, NT]),
            pr[:, :].unsqueeze(2).broadcast_to([128, P, NT]),
            op=Alu.is_ge,
        )
        cnt_pp = bis_pool.tile([128, P], F32, tag="cntpp")
        nc.vector.tensor_reduce(
            cnt_pp[:, :], cmp[:, :].rearrange("p (j t) -> p j t", j=P),
            op=Alu.add, axis=mybir.AxisListType.X)
        cnt = bis_pool.tile([128, P], F32, tag="cnt")
        nc.gpsimd.partition_all_reduce(
            cnt[:, :], cnt_pp[:, :], 128, bass_isa.ReduceOp.add)
        ge = bis_pool.tile([128, P], F32, tag="ge")
        nc.vector.tensor_scalar(ge[:, :], cnt[:, :], kf, None, op0=Alu.is_ge)
        m = bis_pool.tile([128, 1], F32, tag="m")
        nc.vector.tensor_reduce(
            m[:, :], ge[:, :], op=Alu.add, axis=mybir.AxisListType.X)
        nc.vector.tensor_scalar(lo[:, :], m[:, :], wd[:, 0:1],
                                lo[:, 0:1], op0=Alu.mult, op1=Alu.add)
        nc.vector.tensor_scalar(wd[:, :], wd[:, :], 1.0 / (P + 1), None,
                                op0=Alu.mult)

    msks = {}

    def emit_mask(s):
        lo, _ = bstate.pop(s)
        msk = stat_pool.tile([128, NT], I32, tag="msk")
        nc.vector.tensor_tensor(
            msk[:, :], gs[s][:, :],
            lo[:, 0:1].broadcast_to([128, NT]), op=Alu.is_ge)
        msks[s] = msk

    def emit_blend_out(s):
        """Blend mask_token into selected rows (DVE) and DMA chunks out."""
        msk = msks[s]
        dst3 = out_d[s].rearrange("(t p) d -> p t d", p=TOKP)
        for c in range(NCH):
            ch = ctx_chunks.pop((s, c))
            # one predicated copy for the whole chunk (mask broadcast over d)
            nc.vector.copy_predicated(
                ch[:, :].rearrange("p (t d) -> p t d", d=D),
                msk[:, c * MCH : (c + 1) * MCH].unsqueeze(2)
                .broadcast_to([128, MCH, D]),
                mtb[:, :].unsqueeze(1).broadcast_to([128, MCH, D]),
            )
            nc.sync.dma_start(
                dst3[:, c * MCH : (c + 1) * MCH, :],
                ch[:, :].rearrange("p (t d) -> p t d", d=D),
            )

    def emit_scores_interleaved(s, bisect_of, round_tiles):
        """Score tiles of sample s with bisection rounds of a prior sample
        (if any) emitted at the given tile indices."""
        rt = set(round_tiles)
        for t in range(NT):
            emit_score_tile(s, t)
            if bisect_of is not None and t in rt:
                emit_bisect_round(bisect_of)

    # --- pipeline ----------------------------------------------------------
    SPREAD = (2, 5, 8, 11, 14, 17, 20, 23)     # rounds paced ~3 tiles apart
    FRONT = (2, 4, 6, 8, 10, 12, 14, 16)       # front-loaded (tail group)

    emit_in(0, 0, NCH)
    emit_in(1, 0, NCH)
    emit_in(2, 0, NCH - 1)

    emit_scores_interleaved(0, None, ())
    emit_gprep(0)
    emit_scores_interleaved(1, 0, SPREAD)
    emit_mask(0)
    emit_blend_out(0)
    emit_in(2, NCH - 1, NCH)
    emit_in(3, 0, NCH - 1)
    emit_gprep(1)
    emit_scores_interleaved(2, 1, SPREAD)
    emit_mask(1)
    emit_blend_out(1)
    emit_in(3, NCH - 1, NCH)
    emit_gprep(2)
    emit_scores_interleaved(3, 2, FRONT)
    emit_mask(2)
    emit_blend_out(2)
    emit_gprep(3)
    for _ in range(BISECT_ITERS):
        emit_bisect_round(3)
    emit_mask(3)
    emit_blend_out(3)


def build(k):
    from contextlib import ExitStack

    nc = bacc.Bacc("TRN2", target_bir_lowering=False, debug=False,
                   num_devices=NCORES)
    ctx_t = nc.dram_tensor("ctx_in", [BPC, N, D], F32, kind="ExternalInput")
    cond_t = nc.dram_tensor("cond_in", [BPC, D], F32, kind="ExternalInput")
    mt_t = nc.dram_tensor("mt_in", [D], F32, kind="ExternalInput")
    out_t = nc.dram_tensor("out", [BPC, N, D], F32, kind="ExternalOutput")
    with tile.TileContext(nc) as tc:
        with ExitStack() as es:
            _kernel_body(es, tc, out_t.ap(), ctx_t.ap(), cond_t.ap(),
                         mt_t.ap(), k)
    nc.compile()
    return nc


_cache = {}


def kernel(ctx_tokens, cond_feat, mask_token, k):
    k = int(k)
    ctx_np = np.ascontiguousarray(np.asarray(ctx_tokens), dtype=np.float32)
    cond_np = np.ascontiguousarray(np.asarray(cond_feat), dtype=np.float32)
    mt_np = np.ascontiguousarray(np.asarray(mask_token), dtype=np.float32)
    assert ctx_np.shape == (B, N, D) and cond_np.shape == (B, D)

    if k not in _cache:
        _cache[k] = build(k)
    nc = _cache[k]

    in_maps = []
    for c in range(NCORES):
        sl = slice(c * BPC, (c + 1) * BPC)
        in_maps.append({
            "ctx_in": np.ascontiguousarray(ctx_np[sl]),
            "cond_in": np.ascontiguousarray(cond_np[sl]),
            "mt_in": mt_np,
        })
    res = bass_utils.run_bass_kernel_spmd(nc, in_maps, core_ids=list(range(NCORES)))
    out = np.concatenate([res.results[c]["out"] for c in range(NCORES)], axis=0)
    return out.astype(np.asarray(ctx_tokens).dtype, copy=False)


if __name__ == "__main__":
    rng = np.random.default_rng(0)
    ctx = rng.standard_normal((B, N, D), dtype=np.float32)
    cond = rng.standard_normal((B, D), dtype=np.float32)
    mt = rng.standard_normal((D,), dtype=np.float32)
    out = kernel(ctx, cond, mt, 2048)
    print(out.shape, out.dtype)


# revision 12
# speedup vs baseline: 1.1463x; 1.1463x over previous
"""Trainium2 Bass kernel for context-attention guided top-k masking.

Computes, per sample b:
    scores[n] = cos(ctx[b,n,:], cond[b,:])   (l2-normalized dot product)
    sel       = top_k(scores, k)
    out[b,n,:] = mask_token if n in sel else ctx[b,n,:]

Strategy (pure data parallel over batch, 4 samples per NeuronCore x 8 cores):
  - DMA is the roofline: 2 x 33.5 MiB (ctx in + out) through the serialized
    DMA engines at 360 GB/s ~= 186 us. Everything else must hide under it.
  - Engine balance per [128,512] tile: products x*cond on GpSimd (Pool),
    sum-of-squares via ACT Square+accum, dot-reduce alternating between
    ACT Copy+accum and DVE tensor_reduce, blend via DVE copy_predicated.
  - Selection needs no explicit top-k: find the k-th largest score per
    sample by multisection (7 probes/round, 8 rounds, no hi tracking:
    the interval width is the exact per-round constant wd) on the
    rank-monotone transform g = dot * rsqrt(max(ss, eps^2)) ==
    score * ||cond|| (positive per-sample constant -> identical ranking),
    then mask = g >= tau. Rounds of sample s are emitted interleaved with
    the score tiles of sample s+1 so the serial probe chain hides under
    streaming compute instead of stalling the DVE sequencer.
  - All DMAs issue from the SP sequencer in an explicit order (ins ahead
    of outs, one-sample lookahead bounded by the 23 chunk buffers) so the
    in-order issue stream never starves the DMA engines nor deadlocks on
    tile-pool buffer reuse.
"""

import numpy as np

import concourse.bacc as bacc
import concourse.mybir as mybir
import concourse.tile as tile
from concourse import bass_isa
from concourse import bass_utils

B, N, D = 32, 4096, 512
NCORES = 8
BPC = B // NCORES          # samples per core
TOKP = 128                 # tokens per tile (partition dim)
NT = N // TOKP             # 32 tiles per sample
F32 = mybir.dt.float32
I32 = mybir.dt.int32
Alu = mybir.AluOpType
Act = mybir.ActivationFunctionType

# multisection: probes p_j = lo + j*wd (j=1..7) shrink the interval 8x per
# round; 8 rounds from +-G_HI0 resolve 2*G_HI0/8^8 = 4.8e-7, below the min
# adjacent-score gap (~7e-6 in g-space). |tau| ~ |kth cos| * ||cond|| stays
# well inside +-4 for any realistic input (cos of the k-th is O(0.1)).
BISECT_ITERS = 8
G_HI0 = 4.0
P = 7

MCH = 4                    # tiles per DMA chunk (1 MiB transfers)
NCH = NT // MCH            # 8 chunks per sample
CHUNK_BUFS = 23            # ~2.9 samples in flight (184 KiB/partition)


def _kernel_body(ctx_stack, tc, out_d, ctx_d, cond_d, mt_d, k):
    nc = tc.nc
    kf = float(k)

    const_pool = ctx_stack.enter_context(tc.tile_pool(name="const", bufs=1))
    ctx_pool = ctx_stack.enter_context(
        tc.tile_pool(name="ctx", bufs=CHUNK_BUFS))
    prod_pool = ctx_stack.enter_context(tc.tile_pool(name="prod", bufs=3))
    sq_pool = ctx_stack.enter_context(
        tc.tile_pool(name="sq", bufs=2, space="PSUM"))
    stat_pool = ctx_stack.enter_context(tc.tile_pool(name="stat", bufs=2))
    bis_pool = ctx_stack.enter_context(tc.tile_pool(name="bis", bufs=1))

    # --- constants ---------------------------------------------------------
    js = const_pool.tile([128, P], F32, tag="js")
    for j in range(P):
        nc.vector.memset(js[:, j : j + 1], float(j + 1))

    # mask_token broadcast to [128, D]; cond_feat broadcast per sample
    # (one DMA each; issued first so they're on-chip before first use).
    mtb = const_pool.tile([128, D], F32, tag="mtb")
    nc.sync.dma_start(mtb[:, :], mt_d.unsqueeze(0).partition_broadcast(128))
    cond_bt = const_pool.tile([128, BPC * D], F32, tag="condb")
    nc.sync.dma_start(
        cond_bt[:, :].rearrange("p (s d) -> p s d", s=BPC),
        cond_d.unsqueeze(0).partition_broadcast(128),
    )

    ctx_chunks = {}

    def emit_in(s, c_lo, c_hi):
        src3 = ctx_d[s].rearrange("(t p) d -> p t d", p=TOKP)
        for c in range(c_lo, c_hi):
            ch = ctx_pool.tile([TOKP, MCH * D], F32, tag="cchunk")
            nc.sync.dma_start(
                ch[:, :].rearrange("p (t d) -> p t d", d=D),
                src3[:, c * MCH : (c + 1) * MCH, :],
            )
            ctx_chunks[(s, c)] = ch

    stats = {}

    def emit_score_tile(s, t):
        """Products on Pool; ss on ACT; dot-reduce alternates ACT/DVE."""
        if t == 0:
            stats[s] = (
                stat_pool.tile([128, NT], F32, tag="dots", name="dots"),
                stat_pool.tile([128, NT], F32, tag="ss", name="ss"),
            )
        dots, ss = stats[s]
        ct = ctx_chunks[(s, t // MCH)][:, (t % MCH) * D : (t % MCH + 1) * D]
        scr = prod_pool.tile([TOKP, D], F32, tag="scr")
        # Pool runs at 0.42 eff (1111ns/tile): cap it at 19 of 32 tiles
        # per sample so it keeps pace with the 23.3us/sample DMA stream and
        # the bisect partition_all_reduce doesn't queue behind its backlog;
        # DVE picks up 13. Dot-reduce ACT 13 / DVE 19 — the kernel tail is
        # set by each engine's CUMULATIVE work before the last sample's
        # scores, so ACT (whose ss pass is immovable) gets only what keeps
        # its total under ~146us.
        if t % 32 in (2, 5, 7, 10, 13, 15, 18, 21, 23, 26, 29, 30, 31):
            nc.vector.tensor_tensor(
                scr, ct, cond_bt[:, s * D : (s + 1) * D], op=Alu.mult)
        else:
            nc.gpsimd.tensor_tensor(
                scr, ct, cond_bt[:, s * D : (s + 1) * D], op=Alu.mult)
        if t % 2 == 0 and t not in (26, 28, 30):
            dsc = sq_pool.tile([TOKP, D], F32, tag="dsc")
            nc.scalar.activation(
                dsc[:, :], scr, Act.Copy, accum_out=dots[:, t : t + 1])
        else:
            nc.vector.tensor_reduce(
                dots[:, t : t + 1], scr[:, :],
                op=Alu.add, axis=mybir.AxisListType.X)
        sq = sq_pool.tile([TOKP, D], F32, tag="sqs")
        nc.scalar.activation(
            sq[:, :], ct, Act.Square, accum_out=ss[:, t : t + 1])

    gs = {}

    def emit_gprep(s):
        """g = dot * rsqrt(max(ss, 1e-12)); rsqrt = ACT-sqrt seed + one
        Newton step so the norm factor is ~1e-10-relative accurate."""
        dots, ss = stats[s]
        g2 = stat_pool.tile([128, NT], F32, tag="g2")
        ssc = stat_pool.tile([128, NT], F32, tag="ssc")
        nc.vector.tensor_scalar(ssc[:, :], ss[:, :], 1e-12, None, op0=Alu.max)
        inv = stat_pool.tile([128, NT], F32, tag="inv")
        nc.vector.reciprocal(inv[:, :], ssc[:, :])
        r0 = stat_pool.tile([128, NT], F32, tag="r0")
        nc.scalar.activation(r0[:, :], inv[:, :], Act.Sqrt)
        t2 = stat_pool.tile([128, NT], F32, tag="t2")
        nc.vector.tensor_tensor(t2[:, :], r0[:, :], r0[:, :], op=Alu.mult)
        nc.vector.tensor_tensor(t2[:, :], t2[:, :], ssc[:, :], op=Alu.mult)
        nc.vector.tensor_scalar(t2[:, :], t2[:, :], -0.5, 1.5,
                                op0=Alu.mult, op1=Alu.add)
        nc.vector.tensor_tensor(t2[:, :], t2[:, :], r0[:, :], op=Alu.mult)
        nc.vector.tensor_tensor(g2[:, :], dots[:, :], t2[:, :], op=Alu.mult)
        gs[s] = g2

    bstate = {}

    def emit_bisect_round(s):
        """One multisection round: probes pr_j = lo + j*wd, count g >= pr_j,
        m = #probes with count >= k, lo += m*wd, wd /= 8. The final lo is
        exact: pr_m and the lo update compute fl(fl(m*wd)+lo) identically.
        All state is partition-replicated [128,.] and the cross-partition
        count goes through gpsimd partition_all_reduce, keeping the serial
        probe chain off the PE (cold-p-state matmul round-trips are slow)."""
        if s not in bstate:
            lo = bis_pool.tile([128, 1], F32, tag="lo")
            wd = bis_pool.tile([128, 1], F32, tag="wd")
            nc.vector.memset(lo[:, :], -G_HI0)
            nc.vector.memset(wd[:, :], 2.0 * G_HI0 / (P + 1))
            bstate[s] = (lo, wd)
        lo, wd = bstate[s]
        g2 = gs[s]
        pr = bis_pool.tile([128, P], F32, tag="pr")
        nc.vector.tensor_scalar(pr[:, :], js[:, :], wd[:, 0:1],
                                lo[:, 0:1], op0=Alu.mult, op1=Alu.add)
        cmp = bis_pool.tile([128, P * NT], F32, tag="cmp")
        nc.vector.tensor_tensor(
            cmp[:, :].rearrange("p (j t) -> p j t", j=P),
            g2[:, :].unsqueeze(1).broadcast_to([128, P, NT]),
            pr[:, :].unsqueeze(2).broadcast_to([128, P, NT]),
            op=Alu.is_ge,
        )
        cnt_pp = bis_pool.tile([128, P], F32, tag="cntpp")
        nc.vector.tensor_reduce(
            cnt_pp[:, :], cmp[:, :].rearrange("p (j t) -> p j t", j=P),
            op=Alu.add, axis=mybir.AxisListType.X)
        cnt = bis_pool.tile([128, P], F32, tag="cnt")
        nc.gpsimd.partition_all_reduce(
            cnt[:, :], cnt_pp[:, :], 128, bass_isa.ReduceOp.add)
        ge = bis_pool.tile([128, P], F32, tag="ge")
        nc.vector.tensor_scalar(ge[:, :], cnt[:, :], kf, None, op0=Alu.is_ge)
        m = bis_pool.tile([128, 1], F32, tag="m")
        nc.vector.tensor_reduce(
            m[:, :], ge[:, :], op=Alu.add, axis=mybir.AxisListType.X)
        nc.vector.tensor_scalar(lo[:, :], m[:, :], wd[:, 0:1],
                                lo[:, 0:1], op0=Alu.mult, op1=Alu.add)
        nc.vector.tensor_scalar(wd[:, :], wd[:, :], 1.0 / (P + 1), None,
                                op0=Alu.mult)

    msks = {}

    def emit_mask(s):
        lo, _ = bstate.pop(s)
        msk = stat_pool.tile([128, NT], I32, tag="msk")
        nc.vector.tensor_tensor(
            msk[:, :], gs[s][:, :],
            lo[:, 0:1].broadcast_to([128, NT]), op=Alu.is_ge)
        msks[s] = msk

    def emit_blend_out(s):
        """Blend mask_token into selected rows (DVE) and DMA chunks out."""
        msk = msks[s]
        dst3 = out_d[s].rearrange("(t p) d -> p t d", p=TOKP)
        for c in range(NCH):
            ch = ctx_chunks.pop((s, c))
            # one predicated copy for the whole chunk (mask broadcast over d)
            nc.vector.copy_predicated(
                ch[:, :].rearrange("p (t d) -> p t d", d=D),
                msk[:, c * MCH : (c + 1) * MCH].unsqueeze(2)
                .broadcast_to([128, MCH, D]),
                mtb[:, :].unsqueeze(1).broadcast_to([128, MCH, D]),
            )
            nc.sync.dma_start(
                dst3[:, c * MCH : (c + 1) * MCH, :],
                ch[:, :].rearrange("p (t d) -> p t d", d=D),
            )

    def emit_scores_interleaved(s, bisect_of, round_tiles, finish_tile=None):
        """Score tiles of sample s with bisection rounds of a prior sample
        (if any) emitted at the given tile indices. The prior sample's mask
        and blends are emitted right after its last round, BEFORE this
        sample's final chunks' dot-reduces: those wait on the last (late)
        input chunk and must not block the blends in the in-order DVE
        stream."""
        rt = set(round_tiles)
        for t in range(NT):
            emit_score_tile(s, t)
            if bisect_of is not None and t in rt:
                emit_bisect_round(bisect_of)
            if bisect_of is not None and t == finish_tile:
                emit_mask(bisect_of)
                emit_blend_out(bisect_of)

    # --- pipeline ----------------------------------------------------------
    SPREAD = (2, 5, 8, 11, 14, 17, 20, 23)     # rounds paced ~3 tiles apart
    FRONT = (2, 4, 6, 8, 10, 12, 14, 16)       # front-loaded (tail group)

    emit_in(0, 0, NCH)
    emit_in(1, 0, NCH)
    emit_in(2, 0, NCH - 1)

    emit_scores_interleaved(0, None, ())
    emit_gprep(0)
    emit_scores_interleaved(1, 0, SPREAD, finish_tile=24)
    emit_in(2, NCH - 1, NCH)
    emit_in(3, 0, NCH - 1)
    emit_gprep(1)
    emit_scores_interleaved(2, 1, SPREAD, finish_tile=24)
    emit_in(3, NCH - 1, NCH)
    emit_gprep(2)
    emit_scores_interleaved(3, 2, FRONT, finish_tile=17)
    emit_gprep(3)
    for _ in range(BISECT_ITERS):
        emit_bisect_round(3)
    emit_mask(3)
    emit_blend_out(3)


def build(k):
    from contextlib import ExitStack

    nc = bacc.Bacc("TRN2", target_bir_lowering=False, debug=False,
                   num_devices=NCORES)
    ctx_t = nc.dram_tensor("ctx_in", [BPC, N, D], F32, kind="ExternalInput")
    cond_t = nc.dram_tensor("cond_in", [BPC, D], F32, kind="ExternalInput")
    mt_t = nc.dram_tensor("mt_in", [D], F32, kind="ExternalInput")
    out_t = nc.dram_tensor("out", [BPC, N, D], F32, kind="ExternalOutput")
    with tile.TileContext(nc) as tc:
        with ExitStack() as es:
            _kernel_body(es, tc, out_t.ap(), ctx_t.ap(), cond_t.ap(),
                         mt_t.ap(), k)
    nc.compile()
    return nc


_cache = {}


def kernel(ctx_tokens, cond_feat, mask_token, k):
    k = int(k)
    ctx_np = np.ascontiguousarray(np.asarray(ctx_tokens), dtype=np.float32)
    cond_np = np.ascontiguousarray(np.asarray(cond_feat), dtype=np.float32)
    mt_np = np.ascontiguousarray(np.asarray(mask_token), dtype=np.float32)
    assert ctx_np.shape == (B, N, D) and cond_np.shape == (B, D)

    if k not in _cache:
        _cache[k] = build(k)
    nc = _cache[k]

    in_maps = []
    for c in range(NCORES):
        sl = slice(c * BPC, (c + 1) * BPC)
        in_maps.append({
            "ctx_in": np.ascontiguousarray(ctx_np[sl]),
            "cond_in": np.ascontiguousarray(cond_np[sl]),
            "mt_in": mt_np,
        })
    res = bass_utils.run_bass_kernel_spmd(nc, in_maps, core_ids=list(range(NCORES)))
    out = np.concatenate([res.results[c]["out"] for c in range(NCORES)], axis=0)
    return out.astype(np.asarray(ctx_tokens).dtype, copy=False)


if __name__ == "__main__":
    rng = np.random.default_rng(0)
    ctx = rng.standard_normal((B, N, D), dtype=np.float32)
    cond = rng.standard_normal((B, D), dtype=np.float32)
    mt = rng.standard_normal((D,), dtype=np.float32)
    out = kernel(ctx, cond, mt, 2048)
    print(out.shape, out.dtype)
